# revision 1
# baseline (speedup 1.0000x reference)
"""Trainium2 Bass kernel: BFP-quantize -> 3x3 conv -> BatchNorm (batch stats) -> ReLU.

Full-input contract: kernel(x, W, gamma, beta) takes the complete arrays
(x [32,256,56,56] f32, W [256,256,3,3] OIHW f32, gamma/beta [256] f32) and
returns the full [32,256,56,56] f32 output.

Distribution: data-parallel over batch, 4 images per core across 8 cores.
BatchNorm statistics (per-channel sum / sum-of-squares) are exchanged with an
AllGather; every core reduces the 8 contributions locally and applies the
identical normalization to its batch shard.

Per-core pipeline (v2, rebalanced for the measured PE law ~198ns + 0.5ns/col):
  1. DMA each image's 128-channel tile into a zero-padded [58,58] SBUF buffer
     (image 0 in 4 row-bands so quantize/conv start early).
  2. BFP block-quantize (blocks of 32 channels share an exponent), engine-split:
     DVE 32x32 stream-transpose (f32, exactness) + magic-number round;
     GPSIMD block abs-max reduce, scale apply, clip, and the final
     step-multiply (bf16 out); DVE transposes back.
  3. Conv = 9 shifted bf16 matmuls x 2 cin-halves accumulated in PSUM,
     8-row chunks (448 cols), chunk-groups of 3 sharing the tap sweep.
  4. Per-chunk PSUM->ybuf fp16 copy (ScalarE); bn_stats reads the fp16 ybuf.
  5. bn_aggr -> per-core (sum, sumsq) -> AllGather (ring pre-warmed by a dummy
     collective at startup) -> local reduce -> global mean/var -> scale/shift.
  6. BN apply + ReLU split across Scalar/Vector/GpSimd, fp16 staging, fp16
     output DMA (host upcasts to f32).
"""

import sys

for _p in ("/opt/trn_rl_repo",):
    if _p not in sys.path:
        sys.path.insert(0, _p)

import numpy as np
import ml_dtypes

from concourse import bass, bacc, tile, mybir
from concourse.bass_utils import run_bass_kernel_spmd

F32 = mybir.dt.float32
BF16 = mybir.dt.bfloat16
FP16 = mybir.dt.float16
I32 = mybir.dt.int32

P = 128
H = W_SP = 56
HP = 58                      # padded row length
SPATIAL = H * W_SP           # 3136
PADLEN = 3368                # 58*58 = 3364 rounded up so tap APs stay in-bounds
QW0, QW1 = 32, 3360          # 128-aligned quantize window covering all data rows
QLEN = QW1 - QW0             # 3328 = 32*104
NBLK = QLEN // 32            # 104
CIN_T = 2                    # 256 channels = 2 partition tiles
COUT_H = 2
TAPS = 9
ROWS_PER_CHUNK = 8
NCHUNK = H // ROWS_PER_CHUNK          # 7
CHUNK_N = ROWS_PER_CHUNK * W_SP       # 448
MAGIC = float(1.5 * 2.0**23)
EXP_MASK = 0x7F800000
EXP_RSUB = float(0x7F000000)          # 2^-e bits = 0x7F000000 - 2^e bits


def build_program(n_cores: int, imgs_per_core: int, dbg: bool = False):
    nc = bacc.Bacc(
        "TRN2", target_bir_lowering=False, debug=False, num_devices=n_cores
    )
    B = imgs_per_core
    x_d = nc.dram_tensor("x", [B, 256, H, W_SP], F32, kind="ExternalInput")
    wt_d = nc.dram_tensor("wt", [TAPS, CIN_T, P, 256], BF16, kind="ExternalInput")
    gb_d = nc.dram_tensor("gb", [P, 4], F32, kind="ExternalInput")
    out_d = nc.dram_tensor("out", [B, 256, H, W_SP], FP16, kind="ExternalOutput")
    if dbg:
        dbg_xq = nc.dram_tensor("dbg_xq", [CIN_T, P, PADLEN], BF16, kind="ExternalOutput")
        dbg_y = nc.dram_tensor("dbg_y", [COUT_H, P, B * SPATIAL], FP16, kind="ExternalOutput")
        dbg_ss = nc.dram_tensor("dbg_ss", [COUT_H, P, 2], F32, kind="ExternalOutput")

    n_count = float(B * SPATIAL)              # per-core samples per channel
    n_total = float(n_cores * B * SPATIAL)    # global samples per channel

    with tile.TileContext(nc) as tc:
        with (
            tc.tile_pool(name="persist", bufs=1) as pp,
            tc.tile_pool(name="xpad", bufs=1) as xpadp,
            tc.tile_pool(name="xqpad", bufs=1) as xqp,
            tc.tile_pool(name="qf32", bufs=5) as qf,
            tc.tile_pool(name="qbf", bufs=3) as qb,
            tc.tile_pool(name="small", bufs=8) as sm,
            tc.tile_pool(name="tiny", bufs=24) as tp,
            tc.tile_pool(name="ostage", bufs=4) as op_,
            tc.tile_pool(name="psum", bufs=8, space="PSUM") as ps_pool,
            tc.tile_pool(name="dram", bufs=4, space="DRAM") as dramp,
        ):
            # ---- persistent tiles ----
            # weights split by cout half: ch0 is needed by the very first
            # matmul (~20us in), ch1 only ~30us later.  ch0 goes first on the
            # Activation hwdge queue (the SP queue carries image-0's x bands).
            wsb = pp.tile([P, TAPS * CIN_T * 256], BF16, tag="wsb")
            wv = wsb[:].rearrange("p (t k o) -> p t k o", t=TAPS, k=CIN_T)
            nc.scalar.dma_start(
                out=wv[:, :, :, 0:P],
                in_=wt_d.ap()[:, :, :, 0:P].transpose([2, 0, 1, 3]),
            )

            gbsb = pp.tile([P, 4], F32, tag="gbsb")
            nc.scalar.dma_start(out=gbsb[:], in_=gb_d.ap())

            ybuf = [
                pp.tile([P, B * SPATIAL], FP16, tag=f"y{ch}", name=f"ybuf{ch}") for ch in range(COUT_H)
            ]
            stats = [
                pp.tile([P, B * NCHUNK * 6], F32, tag=f"st{ch}", name=f"stats{ch}")
                for ch in range(COUT_H)
            ]

            # fixed padded buffers (pad regions stay zero across image reuse)
            xpad = [xpadp.tile([P, PADLEN], F32, tag=f"xp{ct}", name=f"xpad{ct}") for ct in range(CIN_T)]
            NPHASE = 2
            xq = [
                [xqp.tile([P, PADLEN], BF16, tag=f"xq{phz}_{ct}", name=f"xqpad{phz}_{ct}") for ct in range(CIN_T)]
                for phz in range(NPHASE)
            ]
            # memsets go on the Vector queue: GpSimd must stay free at program
            # start so the collective warm-up (GpSimd-queue, blocking) overlaps
            # the initial DMA instead of stalling the first quantize window
            for t in xpad:
                # zero only the pad positions (head row + per-row col pairs +
                # tail); the interior is overwritten by every image's DMA
                nc.vector.memset(t[:, 0:59], 0.0)
                nc.vector.memset(
                    t[:, 115:115 + 55 * HP].rearrange(
                        "p (r w) -> p r w", r=55
                    )[:, :, 0:2],
                    0.0,
                )
                nc.vector.memset(t[:, 3305:PADLEN], 0.0)
            for phz in range(NPHASE):
                for t in xq[phz]:
                    # quantize overwrites [QW0, QW1) every image; only edges
                    # need the one-time zero fill
                    nc.vector.memset(t[:, :QW0], 0.0)
                    nc.vector.memset(t[:, QW1:], 0.0)

            # preload the sqrt ACT table set so the BN tail doesn't pay it
            warm = tp.tile([P, 1], F32, tag="t1", name="warm")
            nc.scalar.activation(
                warm[:], gbsb[:, 0:1], mybir.ActivationFunctionType.Sqrt
            )

            # one dummy AllGather pays the collective cold-start cost up front
            cc_win = dramp.tile([P, 2 * COUT_H], F32)
            cc_wout = dramp.tile([n_cores, P, 2 * COUT_H], F32)
            nc.scalar.dma_start(out=cc_win[:], in_=gbsb[:])
            nc.gpsimd.collective_compute(
                "AllGather",
                mybir.AluOpType.bypass,
                replica_groups=[list(range(n_cores))],
                ins=[cc_win[:].opt()],
                outs=[cc_wout[:].rearrange("s p v -> (s p v)").unsqueeze(0)],
            )

            dst_interior = lambda t: t[:, HP : HP + 57 * HP].rearrange(
                "p (r w) -> p r w", r=57
            )[:, :H, 1 : 1 + W_SP]

            def quantize_window(phz, w0, wlen):
                """Quantize xpad[*] window [w0, w0+wlen) into xq[phz][*].

                Both cin tiles share one set of small block-param ops.
                Engine split: DVE transposes + magic-round; GPSIMD reduce,
                scale, clip, final step-multiply.
                """
                nb = wlen // 32
                Ts = []
                for ct in range(CIN_T):
                    T = qf.tile([P, wlen], F32, tag="q", name=f"qT{ct}")
                    nc.vector.transpose(T[:], xpad[ct][:, w0 : w0 + wlen])
                    Ts.append(T)
                S = sm.tile([P, 2 * nb], F32, tag="s", name="qS")
                for ct in range(CIN_T):
                    nc.vector.tensor_reduce(
                        S[:, ct * nb : (ct + 1) * nb],
                        Ts[ct][:].rearrange("p (b k) -> p b k", k=32),
                        axis=mybir.AxisListType.X,
                        op=mybir.AluOpType.max,
                        apply_absolute_value=True,
                    )
                m = sm.tile([P, 2 * nb], F32, tag="s", name="qm")
                nc.vector.tensor_scalar(
                    m[:], S[:], 1e-12, None, op0=mybir.AluOpType.max
                )
                peb = sm.tile([P, 2 * nb], I32, tag="s", name="qpeb")
                nc.vector.tensor_scalar(
                    peb[:], m[:].bitcast(I32), EXP_MASK, None,
                    op0=mybir.AluOpType.bitwise_and,
                )
                invb = sm.tile([P, 2 * nb], I32, tag="s", name="qinvb")
                nc.vector.tensor_scalar(
                    invb[:], peb[:], EXP_RSUB, -1.0,
                    op0=mybir.AluOpType.subtract, op1=mybir.AluOpType.mult,
                )
                inv2 = sm.tile([P, 2 * nb], F32, tag="s", name="qinv2")
                nc.vector.tensor_scalar(
                    inv2[:], invb[:].bitcast(F32), 128.0, None,
                    op0=mybir.AluOpType.mult,
                )
                pes = sm.tile([P, 2 * nb], F32, tag="s", name="qpes")
                nc.vector.tensor_scalar(
                    pes[:], peb[:].bitcast(F32), 0.0078125, None,
                    op0=mybir.AluOpType.mult,
                )
                for ct in range(CIN_T):
                    i2 = inv2[:, ct * nb : (ct + 1) * nb]
                    pe = pes[:, ct * nb : (ct + 1) * nb]
                    v = qf.tile([P, wlen], F32, tag="q", name="qv")
                    nc.gpsimd.tensor_tensor(
                        out=v[:].rearrange("p (b k) -> p b k", k=32),
                        in0=Ts[ct][:].rearrange("p (b k) -> p b k", k=32),
                        in1=i2.unsqueeze(2).to_broadcast((P, nb, 32)),
                        op=mybir.AluOpType.mult,
                    )
                    # round-to-nearest-even in ONE dual-op tensor_scalar: the
                    # (v + M) intermediate rounds to fp32 before (- M) applies.
                    # Must stay on Vector: GpSimd's dual ADD,ADD is a
                    # microcoded slow path (~20x, measured 704us total) even
                    # though its fp32 rounding is bit-exact RNE.
                    r2 = qf.tile([P, wlen], F32, tag="q", name="qr2")
                    nc.vector.tensor_scalar(
                        r2[:], v[:], MAGIC, -MAGIC,
                        op0=mybir.AluOpType.add, op1=mybir.AluOpType.add,
                    )
                    c = qf.tile([P, wlen], F32, tag="q", name="qc")
                    nc.gpsimd.tensor_scalar(
                        c[:], r2[:], 127.0, -128.0,
                        op0=mybir.AluOpType.min, op1=mybir.AluOpType.max,
                    )
                    qT = qb.tile([P, wlen], BF16, tag="qb", name="qq")
                    nc.gpsimd.tensor_tensor(
                        out=qT[:].rearrange("p (b k) -> p b k", k=32),
                        in0=c[:].rearrange("p (b k) -> p b k", k=32),
                        in1=pe.unsqueeze(2).to_broadcast((P, nb, 32)),
                        op=mybir.AluOpType.mult,
                    )
                    nc.vector.transpose(xq[phz][ct][:, w0 : w0 + wlen], qT[:])

            # ---- main loop over images (software-pipelined) ----
            # quantize windows are 1:1 with DMA bands (row-16 boundaries are
            # 32-aligned: 58*16=928): window j reads ONLY band j's rows, so
            # each (band-DMA -> window -> conv) chain pipelines independently
            # with a full image-period of slack instead of a just-in-time
            # cascade through the shared xpad buffer.
            HALVES = [896, 928, 928, 576]
            DMA_BANDS = [(0, 15), (15, 31), (31, 47), (47, 56)]
            # image 0: band/window 1 sub-split so conv chunk 0 (grid rows <=9,
            # flat < 580) starts after window 1a alone.  (Finer chunk-aligned
            # windows were tried and measured worse: quantize(0) is Vector-
            # throughput-bound, so extra windows only add per-window overhead.)
            QUARTERS = [576, 320, 928, 928, 576]
            DMA_BANDS0 = [(0, 10), (10, 15), (15, 31), (31, 47), (47, 56)]

            def emit_dma_img(img, split_queues=False):
                # banded so the write-after-read on xpad is fine-grained:
                # band k only waits for the previous image's windows that
                # actually read those rows.  The hwdge queue serializes
                # transfers, so image 0 splits its two cin tiles across the
                # SP and Activation queues to halve the head DMA time.
                bands = DMA_BANDS0 if split_queues else DMA_BANDS
                for ct in range(CIN_T):
                    xp = xpad[ct]
                    src = x_d.ap()[img, ct * P : (ct + 1) * P]
                    eng = nc.scalar if (split_queues and ct == 1) else nc.sync
                    for r0, r1 in bands:
                        eng.dma_start(
                            out=dst_interior(xp)[:, r0:r1, :],
                            in_=src[:, r0:r1, :].rearrange("c h w -> c (h w)"),
                        )

            def emit_quantize(img, windows):
                phz = img % NPHASE
                off = QW0
                for wlen in windows:
                    quantize_window(phz, off, wlen)
                    off += wlen

            def emit_conv(img, chs=(0, 1)):
                phz = img % NPHASE
                # chunk-outer: weights self-load per matmul anyway, and a
                # single 18-deep accumulation chain per chunk gives the
                # earliest possible start for each chunk's first matmul
                for ch in chs:
                    for chunk in range(NCHUNK):
                        ps = ps_pool.tile(
                            [P, CHUNK_N], F32, tag="ps", name=f"ps{chunk}"
                        )
                        # kt-major: all cin-half-0 taps first, so the second
                        # cin tile's quantize latency hides under kt0 matmuls
                        for kt in range(CIN_T):
                            for tap in range(TAPS):
                                kh, kw = divmod(tap, 3)
                                acc_i = kt * TAPS + tap
                                lhsT = wv[:, tap, kt, ch * P : (ch + 1) * P]
                                base = (chunk * ROWS_PER_CHUNK + kh) * HP + kw
                                rhs = (
                                    xq[phz][kt][
                                        :, base : base + ROWS_PER_CHUNK * HP
                                    ]
                                    .rearrange(
                                        "p (r w) -> p r w", r=ROWS_PER_CHUNK
                                    )[:, :, :W_SP]
                                )
                                nc.tensor.matmul(
                                    ps[:],
                                    lhsT,
                                    rhs,
                                    start=(acc_i == 0),
                                    stop=(acc_i == 2 * TAPS - 1),
                                )
                        ysl = ybuf[ch][
                            :, img * SPATIAL + chunk * CHUNK_N :
                            img * SPATIAL + (chunk + 1) * CHUNK_N
                        ]
                        nc.scalar.activation(
                            ysl, ps[:],
                            mybir.ActivationFunctionType.Copy,
                        )
                        k6 = (img * NCHUNK + chunk) * 6
                        nc.vector.bn_stats(
                            stats[ch][:, k6 : k6 + 6], ysl
                        )

            emit_dma_img(0, split_queues=True)
            # second weight half arrives behind image 0's ct0 bands
            nc.sync.dma_start(
                out=wv[:, :, :, P : 2 * P],
                in_=wt_d.ap()[:, :, :, P : 2 * P].transpose([2, 0, 1, 3]),
            )
            emit_quantize(0, QUARTERS)  # fine-grained: shortens the head

            for img in range(B):
                if img + 1 < B:
                    emit_dma_img(img + 1)
                    emit_quantize(img + 1, HALVES)
                emit_conv(img)

            # ---- BN statistics exchange (AllGather + local reduce) ----
            sums_all = pp.tile([P, 2 * COUT_H], F32, tag="sums_all")
            for ch in range(COUT_H):
                mv = tp.tile([P, 2], F32, tag="t2")
                nc.vector.bn_aggr(
                    mv[:], stats[ch][:].rearrange("p (n s) -> p n s", s=6)
                )
                mean2 = tp.tile([P, 1], F32, tag="t1")
                nc.vector.tensor_tensor(
                    mean2[:], mv[:, 0:1], mv[:, 0:1], op=mybir.AluOpType.mult
                )
                nc.vector.tensor_scalar(
                    sums_all[:, 2 * ch : 2 * ch + 1], mv[:, 0:1], n_count, None,
                    op0=mybir.AluOpType.mult,
                )
                nc.vector.tensor_scalar(
                    sums_all[:, 2 * ch + 1 : 2 * ch + 2], mv[:, 1:2],
                    mean2[:, 0:1], n_count,
                    op0=mybir.AluOpType.add, op1=mybir.AluOpType.mult,
                )
            cc_in = dramp.tile([P, 2 * COUT_H], F32)
            # AllGather concatenates the ranks' flat [P, 4] buffers: [s, p, v]
            cc_out = dramp.tile([n_cores, P, 2 * COUT_H], F32)
            nc.sync.dma_start(out=cc_in[:], in_=sums_all[:])
            nc.gpsimd.collective_compute(
                "AllGather",
                mybir.AluOpType.bypass,
                replica_groups=[list(range(n_cores))],
                ins=[cc_in[:].opt()],
                outs=[cc_out[:].rearrange("s p v -> (s p v)").unsqueeze(0)],
            )
            allsums = pp.tile([P, n_cores * 2 * COUT_H], F32, tag="allsums")
            nc.sync.dma_start(
                out=allsums[:].rearrange("p (s v) -> p s v", s=n_cores),
                in_=cc_out[:].transpose([1, 0, 2]),
            )
            gsum = tp.tile([P, 2 * COUT_H], F32, tag="t4", name="gsum")
            nc.vector.tensor_reduce(
                gsum[:],
                allsums[:].rearrange("p (s v) -> p v s", s=n_cores),
                axis=mybir.AxisListType.X,
                op=mybir.AluOpType.add,
            )
            # BN math on both channel halves at once ([P, 2]-wide ops)
            g2 = gsum[:].rearrange("p (c v) -> p c v", v=2)
            gmean = tp.tile([P, 2], F32, tag="t2")
            nc.vector.tensor_scalar(
                gmean[:], g2[:, :, 0], 1.0 / n_total, None,
                op0=mybir.AluOpType.mult,
            )
            gex2 = tp.tile([P, 2], F32, tag="t2")
            nc.vector.tensor_scalar(
                gex2[:], g2[:, :, 1], 1.0 / n_total, None,
                op0=mybir.AluOpType.mult,
            )
            gm2 = tp.tile([P, 2], F32, tag="t2")
            nc.vector.tensor_tensor(
                gm2[:], gmean[:], gmean[:], op=mybir.AluOpType.mult
            )
            veps = tp.tile([P, 2], F32, tag="t2")  # var + eps
            nc.vector.scalar_tensor_tensor(
                veps[:], gex2[:], 1e-5, gm2[:],
                op0=mybir.AluOpType.add, op1=mybir.AluOpType.subtract,
            )
            rec = tp.tile([P, 2], F32, tag="t2")  # 1/(var+eps)
            nc.vector.reciprocal(rec[:], veps[:])
            s0 = tp.tile([P, 2], F32, tag="t2")  # ~= 1/sqrt(var+eps)
            nc.scalar.activation(
                s0[:], rec[:], mybir.ActivationFunctionType.Sqrt
            )
            # one Newton step: s1 = s0 * (1.5 - 0.5 * veps * s0^2)
            a = tp.tile([P, 2], F32, tag="t2")
            nc.vector.tensor_tensor(a[:], s0[:], s0[:], op=mybir.AluOpType.mult)
            b = tp.tile([P, 2], F32, tag="t2")
            nc.vector.tensor_tensor(b[:], a[:], veps[:], op=mybir.AluOpType.mult)
            bb = tp.tile([P, 2], F32, tag="t2")
            nc.vector.tensor_scalar(
                bb[:], b[:], -0.5, 1.5,
                op0=mybir.AluOpType.mult, op1=mybir.AluOpType.add,
            )
            s1 = tp.tile([P, 2], F32, tag="t2")
            nc.vector.tensor_tensor(s1[:], s0[:], bb[:], op=mybir.AluOpType.mult)
            scale2 = tp.tile([P, 2], F32, tag="sc2")
            nc.vector.tensor_tensor(
                scale2[:], s1[:], gbsb[:, 0:2], op=mybir.AluOpType.mult
            )
            t2m = tp.tile([P, 2], F32, tag="t2")
            nc.vector.tensor_tensor(
                t2m[:], gmean[:], scale2[:], op=mybir.AluOpType.mult
            )
            shift2 = tp.tile([P, 2], F32, tag="sc2")
            nc.vector.scalar_tensor_tensor(
                shift2[:], t2m[:], -1.0, gbsb[:, 2:4],
                op0=mybir.AluOpType.mult, op1=mybir.AluOpType.add,
            )
            scales = [scale2[:, 0:1], scale2[:, 1:2]]
            shifts = [shift2[:, 0:1], shift2[:, 1:2]]

            if dbg:
                for ct in range(CIN_T):
                    nc.sync.dma_start(out=dbg_xq.ap()[ct], in_=xq[0][ct][:])
                for ch in range(COUT_H):
                    nc.sync.dma_start(out=dbg_y.ap()[ch], in_=ybuf[ch][:])
                    ss = tp.tile([P, 2], F32, tag="t2", name=f"dss{ch}")
                    nc.vector.tensor_copy(ss[:, 0:1], scales[ch])
                    nc.vector.tensor_copy(ss[:, 1:2], shifts[ch])
                    nc.sync.dma_start(out=dbg_ss.ap()[ch], in_=ss[:])

            # ---- apply BN + ReLU, write out (fp16 DMA).
            # ScalarE fused ACT for most units; Vector takes the first two
            # via broadcast tensor_tensor + const-max (the AP-scalar
            # tensor_scalar and any GpSimd fp16 form are microcoded slow
            # paths, 40+us per op -- never use those here).
            for u, (img, ch) in enumerate(
                (i, c) for i in range(B) for c in range(COUT_H)
            ):
                ysl = ybuf[ch][:, img * SPATIAL : (img + 1) * SPATIAL]
                o = op_.tile([P, SPATIAL], FP16, tag="o", name="ostage")
                if u < 2:
                    # staging from the (tail-idle) qf pool: its slot size
                    # already covers [P, SPATIAL] fp16, so the SBUF layout is
                    # unchanged (layout shifts re-roll the placement lottery)
                    t1 = qf.tile([P, SPATIAL], FP16, tag="q", name="apply_t1")
                    nc.vector.tensor_tensor(
                        out=t1[:], in0=ysl,
                        in1=scales[ch].to_broadcast((P, SPATIAL)),
                        op=mybir.AluOpType.mult,
                    )
                    t2 = qf.tile([P, SPATIAL], FP16, tag="q", name="apply_t2")
                    nc.vector.tensor_tensor(
                        out=t2[:], in0=t1[:],
                        in1=shifts[ch].to_broadcast((P, SPATIAL)),
                        op=mybir.AluOpType.add,
                    )
                    nc.vector.tensor_scalar(
                        o[:], t2[:], 0.0, None, op0=mybir.AluOpType.max
                    )
                else:
                    nc.scalar.activation(
                        o[:], ysl,
                        mybir.ActivationFunctionType.Relu,
                        bias=shifts[ch],
                        scale=scales[ch],
                    )
                nc.sync.dma_start(
                    out=out_d.ap()[img, ch * P : (ch + 1) * P].rearrange(
                        "c h w -> c (h w)"
                    ),
                    in_=o[:],
                )

    nc.compile()
    return nc


def host_prep(W, gamma, beta):
    # lhsT layout per tap: [cin, cout];  wt[t, kt, p, o] = W[o, kt*128+p, kh, kw]
    wt = np.ascontiguousarray(
        W.transpose(2, 3, 1, 0).reshape(TAPS, CIN_T, P, 256)
    ).astype(ml_dtypes.bfloat16)
    gb = np.empty((P, 4), np.float32)
    gb[:, 0] = gamma[:P]
    gb[:, 1] = gamma[P:]
    gb[:, 2] = beta[:P]
    gb[:, 3] = beta[P:]
    return wt, gb


_cache = {}


def _get_program(n_cores, imgs_per_core):
    key = (n_cores, imgs_per_core)
    if key not in _cache:
        _cache[key] = build_program(n_cores, imgs_per_core)
    return _cache[key]


def run(x, W, gamma, beta, n_cores=8, trace=False):
    B = x.shape[0]
    imgs_per_core = B // n_cores
    assert imgs_per_core * n_cores == B
    nc = _get_program(n_cores, imgs_per_core)
    wt, gb = host_prep(W, gamma, beta)
    in_maps = [
        {
            "x": np.ascontiguousarray(
                x[c * imgs_per_core : (c + 1) * imgs_per_core]
            ),
            "wt": wt,
            "gb": gb,
        }
        for c in range(n_cores)
    ]
    res = run_bass_kernel_spmd(nc, in_maps, list(range(n_cores)), trace=trace)
    out = np.concatenate(
        [res.results[c]["out"].astype(np.float32) for c in range(n_cores)], axis=0
    )
    return out, res


def kernel(x, W, gamma, beta):
    out, _ = run(
        np.asarray(x, np.float32),
        np.asarray(W, np.float32),
        np.asarray(gamma, np.float32),
        np.asarray(beta, np.float32),
    )
    return out



# revision 2
# speedup vs baseline: 1.0294x; 1.0294x over previous
"""Trainium2 Bass kernel v3: BFP-quantize -> 3x3 conv -> BatchNorm -> ReLU.

Full-input contract: kernel(x, W, gamma, beta) takes the complete arrays
(x [32,256,56,56] f32, W [256,256,3,3] OIHW f32, gamma/beta [256] f32) and
returns the full [32,256,56,56] f32 output.

Distribution: data-parallel over batch, 4 images per core across 8 cores.
BatchNorm statistics are AllReduce-summed per cout-half so the first half's
collective overlaps the second half's conv matmuls.

Structure (from baseline trace analysis):
  1. Dense input DMA: x lands contiguously in SBUF ([128, 3136] per cin-tile)
     instead of scattering 224B rows into a padded buffer (57k descriptors).
     The zero-padded 58-wide layout is produced by the quantize pipeline's
     final DVE transpose-back through a strided [p, rows, 56(@58)] dst AP
     (verified bit-exact on HW).
  2. Image-0 head: bands split across both DMA queues; window 0 quantize is
     per-cin-tile (ct0 fully on Vector first, ct1's big ops on GpSimd) so the
     first matmul starts as early as possible.  The collective warm-up is
     emitted after image 2's quantize so it never blocks the GpSimd queue
     when quantize needs it.
  3. bn_stats per image (one [P,3136] op) except the last image's second
     cout-half, which stays per-chunk so the final stats land right behind
     the last PSUM copy.
  4. Tail: per-cout-half AllReduce; half 0's exchange + BN apply interleave
     into half 1's conv emission.  The GpSimd queue carries nothing between
     the two collectives (an out-DMA post there would delay cc1 behind a
     late Vector apply).  Each apply unit's fp16 out-DMA is split in half
     across two queues.
"""

import sys

for _p in ("/opt/trn_rl_repo",):
    if _p not in sys.path:
        sys.path.insert(0, _p)

import numpy as np
import ml_dtypes

from concourse import bass, bacc, tile, mybir
from concourse.bass_utils import run_bass_kernel_spmd

F32 = mybir.dt.float32
BF16 = mybir.dt.bfloat16
FP16 = mybir.dt.float16
I32 = mybir.dt.int32

P = 128
H = W_SP = 56
HP = 58                      # padded row length
SPATIAL = H * W_SP           # 3136
HALFSP = SPATIAL // 2
PADLEN = 3368                # 58*58 = 3364 rounded up so tap APs stay in-bounds
CIN_T = 2                    # 256 channels = 2 partition tiles
COUT_H = 2
TAPS = 9
ROWS_PER_CHUNK = 8
NCHUNK = H // ROWS_PER_CHUNK          # 7
CHUNK_N = ROWS_PER_CHUNK * W_SP       # 448
MAGIC = float(1.5 * 2.0**23)
EXP_MASK = 0x7F800000
EXP_RSUB = float(0x7F000000)          # 2^-e bits = 0x7F000000 - 2^e bits

# dense-row bands; each is both a DMA band and a quantize window.
# nrows must be a multiple of 4 so the window length (nrows*56) is a
# multiple of 32 (DVE 32x32 stream-transpose blocks).
BANDS = [(0, 12), (12, 28), (28, 44), (44, 56)]

# stats group slots: per (img, chunk); bn_stats free size is HW-capped at 512
NST = [4 * NCHUNK, 4 * NCHUNK]


def build_program(n_cores: int, imgs_per_core: int, dbg: bool = False):
    nc = bacc.Bacc(
        "TRN2", target_bir_lowering=False, debug=False, num_devices=n_cores
    )
    B = imgs_per_core
    x_d = nc.dram_tensor("x", [B, 256, H, W_SP], F32, kind="ExternalInput")
    wt_d = nc.dram_tensor("wt", [COUT_H, P, TAPS * CIN_T * P], BF16, kind="ExternalInput")
    gb_d = nc.dram_tensor("gb", [P, 4], F32, kind="ExternalInput")
    out_d = nc.dram_tensor("out", [B, 256, H, W_SP], FP16, kind="ExternalOutput")
    if dbg:
        dbg_xq = nc.dram_tensor("dbg_xq", [CIN_T, P, PADLEN], BF16, kind="ExternalOutput")
        dbg_y = nc.dram_tensor("dbg_y", [COUT_H, P, B * SPATIAL], FP16, kind="ExternalOutput")
        dbg_ss = nc.dram_tensor("dbg_ss", [COUT_H, P, 2], F32, kind="ExternalOutput")

    n_count = float(B * SPATIAL)              # per-core samples per channel
    n_total = float(n_cores * B * SPATIAL)    # global samples per channel

    with tile.TileContext(nc) as tc:
        with (
            tc.tile_pool(name="persist", bufs=1) as pp,
            tc.tile_pool(name="xpad", bufs=1) as xpadp,
            tc.tile_pool(name="xqpad", bufs=1) as xqp,
            tc.tile_pool(name="qf32", bufs=8) as qf,
            tc.tile_pool(name="qbf", bufs=3) as qb,
            tc.tile_pool(name="small", bufs=10) as sm,
            tc.tile_pool(name="tiny", bufs=24) as tp,
            tc.tile_pool(name="bnm", bufs=28) as bnp,
            tc.tile_pool(name="ostage", bufs=4) as op_,
            tc.tile_pool(name="vtmp", bufs=4) as vt,
            tc.tile_pool(name="psum", bufs=8, space="PSUM") as ps_pool,
            tc.tile_pool(name="dram", bufs=8, space="DRAM") as dramp,
        ):
            # ---- persistent tiles ----
            wsb = pp.tile([P, COUT_H * TAPS * CIN_T * P], BF16, tag="wsb")
            wv = wsb[:].rearrange(
                "p (c t k o) -> p c t k o", c=COUT_H, t=TAPS, k=CIN_T
            )
            gbsb = pp.tile([P, 4], F32, tag="gbsb")
            # weights + gamma/beta on the GpSimd hwdge queue (sync/scalar
            # queues carry image-0's x bands)
            nc.gpsimd.dma_start(out=gbsb[:], in_=gb_d.ap())
            nc.gpsimd.dma_start(
                out=wsb[:, 0 : TAPS * CIN_T * P], in_=wt_d.ap()[0]
            )
            nc.gpsimd.dma_start(
                out=wsb[:, TAPS * CIN_T * P :], in_=wt_d.ap()[1]
            )

            ybuf = [
                pp.tile([P, B * SPATIAL], FP16, tag=f"y{ch}", name=f"ybuf{ch}")
                for ch in range(COUT_H)
            ]
            stats = [
                pp.tile([P, NST[ch] * 6], F32, tag=f"st{ch}", name=f"stats{ch}")
                for ch in range(COUT_H)
            ]
            sums = [
                pp.tile([P, 2], F32, tag=f"sm{ch}", name=f"sums{ch}")
                for ch in range(COUT_H)
            ]
            allred = [
                pp.tile([P, 2], F32, tag=f"ar{ch}", name=f"allred{ch}")
                for ch in range(COUT_H)
            ]

            # dense activation staging (fully overwritten per image; no memset)
            xpad = [
                xpadp.tile([P, SPATIAL], F32, tag=f"xp{ct}", name=f"xpad{ct}")
                for ct in range(CIN_T)
            ]
            NPHASE = 2
            xq = [
                [
                    xqp.tile([P, PADLEN], BF16, tag=f"xq{phz}_{ct}", name=f"xqpad{phz}_{ct}")
                    for ct in range(CIN_T)
                ]
                for phz in range(NPHASE)
            ]
            # zero only the pad positions; the interior is overwritten by
            # every image's quantize transpose-back
            for phz in range(NPHASE):
                for t in xq[phz]:
                    nc.vector.memset(t[:, 0:59], 0.0)
                    nc.vector.memset(
                        t[:, 115 : 115 + 55 * HP].rearrange(
                            "p (r w) -> p r w", r=55
                        )[:, :, 0:2],
                        0.0,
                    )
                    nc.vector.memset(t[:, 3305:PADLEN], 0.0)

            # preload the sqrt ACT table set so the BN tail doesn't pay it
            warm = tp.tile([P, 1], F32, tag="t1", name="warm")
            nc.scalar.activation(
                warm[:], gbsb[:, 0:1], mybir.ActivationFunctionType.Sqrt
            )
            # spin up the GpSimd Q7s on a dummy op so image 0's window-1
            # big ops don't pay the cold first-op latency
            warmg = tp.tile([P, 4], F32, tag="t4", name="warmg")
            nc.gpsimd.memset(warmg[:], 0.0)

            cc_win = dramp.tile([P, 2], F32, name="cc_win")
            cc_wout = dramp.tile([P, 2], F32, name="cc_wout")
            nc.scalar.dma_start(out=cc_win[:], in_=gbsb[:, 0:2])
            cc_in = [
                dramp.tile([P, 2], F32, name=f"cc_in{ch}") for ch in range(COUT_H)
            ]
            cc_out = [
                dramp.tile([P, 2], F32, name=f"cc_out{ch}") for ch in range(COUT_H)
            ]

            def big_ops(ct, eng, Ts, i2, pe, nb, wlen, phz, r0, r1):
                """Per-cin-tile scale -> round -> clip -> step-multiply ->
                transpose-back (dense rows scattered into the padded layout)."""
                v = qf.tile([P, wlen], F32, tag="q", name="qv")
                eng.tensor_tensor(
                    out=v[:].rearrange("p (b k) -> p b k", k=32),
                    in0=Ts[ct][:].rearrange("p (b k) -> p b k", k=32),
                    in1=i2.unsqueeze(2).to_broadcast((P, nb, 32)),
                    op=mybir.AluOpType.mult,
                )
                # round-to-nearest-even in ONE dual-op tensor_scalar: the
                # (v + M) intermediate rounds to fp32 before (- M) applies.
                # Must stay on Vector: GpSimd's dual ADD,ADD is a microcoded
                # slow path (~20x) even though its fp32 rounding is RNE.
                r2 = qf.tile([P, wlen], F32, tag="q", name="qr2")
                nc.vector.tensor_scalar(
                    r2[:], v[:], MAGIC, -MAGIC,
                    op0=mybir.AluOpType.add, op1=mybir.AluOpType.add,
                )
                c = qf.tile([P, wlen], F32, tag="q", name="qc")
                eng.tensor_scalar(
                    c[:], r2[:], 127.0, -128.0,
                    op0=mybir.AluOpType.min, op1=mybir.AluOpType.max,
                )
                qT = qb.tile([P, wlen], BF16, tag="qb", name="qq")
                eng.tensor_tensor(
                    out=qT[:].rearrange("p (b k) -> p b k", k=32),
                    in0=c[:].rearrange("p (b k) -> p b k", k=32),
                    in1=pe.unsqueeze(2).to_broadcast((P, nb, 32)),
                    op=mybir.AluOpType.mult,
                )
                dst = xq[phz][ct][
                    :, HP * (r0 + 1) + 1 : HP * (r0 + 1) + 1 + (r1 - r0) * HP
                ].rearrange("p (r w) -> p r w", w=HP)[:, :, :W_SP]
                nc.vector.transpose(dst, qT[:])

            def small_chain(S):
                """Block max -> (1/step, step) via exponent bit tricks."""
                n = S.shape[1]
                m = sm.tile([P, n], F32, tag="s", name="qm")
                nc.vector.tensor_scalar(
                    m[:], S, 1e-12, None, op0=mybir.AluOpType.max
                )
                peb = sm.tile([P, n], I32, tag="s", name="qpeb")
                nc.vector.tensor_scalar(
                    peb[:], m[:].bitcast(I32), EXP_MASK, None,
                    op0=mybir.AluOpType.bitwise_and,
                )
                invb = sm.tile([P, n], I32, tag="s", name="qinvb")
                nc.vector.tensor_scalar(
                    invb[:], peb[:], EXP_RSUB, -1.0,
                    op0=mybir.AluOpType.subtract, op1=mybir.AluOpType.mult,
                )
                inv2 = sm.tile([P, n], F32, tag="s", name="qinv2")
                nc.vector.tensor_scalar(
                    inv2[:], invb[:].bitcast(F32), 128.0, None,
                    op0=mybir.AluOpType.mult,
                )
                pes = sm.tile([P, n], F32, tag="s", name="qpes")
                nc.vector.tensor_scalar(
                    pes[:], peb[:].bitcast(F32), 0.0078125, None,
                    op0=mybir.AluOpType.mult,
                )
                return inv2, pes

            def quantize_window(phz, r0, r1, big_engs, split_small=False):
                """Quantize dense rows [r0, r1) of xpad[*] into the padded
                58-wide interior of xq[phz][*]."""
                w0 = r0 * W_SP
                wlen = (r1 - r0) * W_SP
                nb = wlen // 32
                if split_small:
                    # fully per-ct chains: ct0 completes before ct1 starts,
                    # so the first conv chunk's kt0 taps can begin earliest
                    for ct in range(CIN_T):
                        T = qf.tile([P, wlen], F32, tag="q", name=f"qT{ct}")
                        nc.vector.transpose(T[:], xpad[ct][:, w0 : w0 + wlen])
                        S = sm.tile([P, nb], F32, tag="s", name="qS")
                        nc.vector.tensor_reduce(
                            S[:],
                            T[:].rearrange("p (b k) -> p b k", k=32),
                            axis=mybir.AxisListType.X,
                            op=mybir.AluOpType.max,
                            apply_absolute_value=True,
                        )
                        inv2, pes = small_chain(S[:])
                        big_ops(
                            ct, big_engs[ct], {ct: T}, inv2[:], pes[:],
                            nb, wlen, phz, r0, r1,
                        )
                else:
                    Ts = []
                    for ct in range(CIN_T):
                        T = qf.tile([P, wlen], F32, tag="q", name=f"qT{ct}")
                        nc.vector.transpose(T[:], xpad[ct][:, w0 : w0 + wlen])
                        Ts.append(T)
                    S = sm.tile([P, 2 * nb], F32, tag="s", name="qS")
                    for ct in range(CIN_T):
                        nc.vector.tensor_reduce(
                            S[:, ct * nb : (ct + 1) * nb],
                            Ts[ct][:].rearrange("p (b k) -> p b k", k=32),
                            axis=mybir.AxisListType.X,
                            op=mybir.AluOpType.max,
                            apply_absolute_value=True,
                        )
                    inv2, pes = small_chain(S[:])
                    for ct in range(CIN_T):
                        big_ops(
                            ct, big_engs[ct], Ts,
                            inv2[:, ct * nb : (ct + 1) * nb],
                            pes[:, ct * nb : (ct + 1) * nb],
                            nb, wlen, phz, r0, r1,
                        )

            def emit_dma_img(img, split_queues=False):
                # dense contiguous bands; band k only waits for the previous
                # image's quantize window k (which reads exactly those rows)
                if split_queues:
                    # image 0: halve each (band, ct) across both queues so
                    # band 0 of both cin tiles lands as early as possible
                    for r0, r1 in BANDS:
                        mid = (r0 + r1) // 2
                        for ct in range(CIN_T):
                            src = x_d.ap()[img, ct * P : (ct + 1) * P]
                            nc.sync.dma_start(
                                out=xpad[ct][:, r0 * W_SP : mid * W_SP],
                                in_=src[:, r0:mid, :].rearrange("c h w -> c (h w)"),
                            )
                            nc.scalar.dma_start(
                                out=xpad[ct][:, mid * W_SP : r1 * W_SP],
                                in_=src[:, mid:r1, :].rearrange("c h w -> c (h w)"),
                            )
                    return
                for ct in range(CIN_T):
                    eng = nc.sync if ct == 0 else nc.scalar
                    src = x_d.ap()[img, ct * P : (ct + 1) * P]
                    for r0, r1 in BANDS:
                        eng.dma_start(
                            out=xpad[ct][:, r0 * W_SP : r1 * W_SP],
                            in_=src[:, r0:r1, :].rearrange("c h w -> c (h w)"),
                        )

            GPS = (None, None)  # placeholder replaced below

            def emit_quantize(img, head=False):
                phz = img % NPHASE
                for wi, (r0, r1) in enumerate(BANDS):
                    if head and wi == 0:
                        # all-Vector: GpSimd's cold first-op latency makes it
                        # a poor choice on the very first window
                        quantize_window(
                            phz, r0, r1, (nc.vector, nc.vector), split_small=True
                        )
                    elif head:
                        # split the two cin tiles across engines so their
                        # chains run in parallel (image 0 has no slack)
                        quantize_window(phz, r0, r1, (nc.vector, nc.gpsimd))
                    else:
                        quantize_window(phz, r0, r1, (nc.gpsimd, nc.gpsimd))

            def emit_chunk(img, ch, chunk, stats_slot=None):
                phz = img % NPHASE
                ps = ps_pool.tile([P, CHUNK_N], F32, tag="ps", name=f"ps{chunk}")
                # kt-major: all cin-half-0 taps first, so the second cin
                # tile's quantize latency hides under kt0 matmuls
                for kt in range(CIN_T):
                    for tap in range(TAPS):
                        kh, kw = divmod(tap, 3)
                        acc_i = kt * TAPS + tap
                        lhsT = wv[:, ch, tap, kt, :]
                        base = (chunk * ROWS_PER_CHUNK + kh) * HP + kw
                        rhs = (
                            xq[phz][kt][:, base : base + ROWS_PER_CHUNK * HP]
                            .rearrange("p (r w) -> p r w", r=ROWS_PER_CHUNK)[
                                :, :, :W_SP
                            ]
                        )
                        nc.tensor.matmul(
                            ps[:],
                            lhsT,
                            rhs,
                            start=(acc_i == 0),
                            stop=(acc_i == 2 * TAPS - 1),
                        )
                ysl = ybuf[ch][
                    :, img * SPATIAL + chunk * CHUNK_N :
                    img * SPATIAL + (chunk + 1) * CHUNK_N
                ]
                nc.scalar.activation(
                    ysl, ps[:], mybir.ActivationFunctionType.Copy
                )
                if stats_slot is not None:
                    k6 = stats_slot * 6
                    nc.vector.bn_stats(stats[ch][:, k6 : k6 + 6], ysl)

            def emit_conv(img):
                for ch in range(COUT_H):
                    for chunk in range(NCHUNK):
                        emit_chunk(img, ch, chunk, stats_slot=img * NCHUNK + chunk)

            def emit_aggr_cc(ch):
                """Local (sum, sumsq) for this cout half + AllReduce."""
                mv = tp.tile([P, 2], F32, tag="t2")
                nc.vector.bn_aggr(
                    mv[:], stats[ch][:].rearrange("p (n s) -> p n s", s=6)
                )
                mean2 = tp.tile([P, 1], F32, tag="t1")
                nc.vector.tensor_tensor(
                    mean2[:], mv[:, 0:1], mv[:, 0:1], op=mybir.AluOpType.mult
                )
                nc.vector.tensor_scalar(
                    sums[ch][:, 0:1], mv[:, 0:1], n_count, None,
                    op0=mybir.AluOpType.mult,
                )
                nc.vector.tensor_scalar(
                    sums[ch][:, 1:2], mv[:, 1:2], mean2[:, 0:1], n_count,
                    op0=mybir.AluOpType.add, op1=mybir.AluOpType.mult,
                )
                # collective-adjacent DMAs must be HWDGE (sync): on the
                # GpSimd queue they are SWDGE, and the DVE applies running
                # between the two collectives lock the shared port, starving
                # Q7 descriptor generation -> +13us skew at cc1's barrier
                nc.sync.dma_start(out=cc_in[ch][:], in_=sums[ch][:])
                nc.gpsimd.collective_compute(
                    "AllReduce",
                    mybir.AluOpType.add,
                    replica_groups=[list(range(n_cores))],
                    ins=[cc_in[ch][:].opt()],
                    outs=[cc_out[ch][:].opt()],
                )
                nc.sync.dma_start(out=allred[ch][:], in_=cc_out[ch][:])

            bnt = {}

            def emit_bn_pre(ch):
                """Vector: global mean / E[x^2] -> var+eps -> 1/(var+eps)."""
                gmean = bnp.tile([P, 1], F32, tag="b1", name="gmean")
                nc.vector.tensor_scalar(
                    gmean[:], allred[ch][:, 0:1], 1.0 / n_total, None,
                    op0=mybir.AluOpType.mult,
                )
                gex2 = bnp.tile([P, 1], F32, tag="b1", name="gex2")
                nc.vector.tensor_scalar(
                    gex2[:], allred[ch][:, 1:2], 1.0 / n_total, None,
                    op0=mybir.AluOpType.mult,
                )
                gm2 = bnp.tile([P, 1], F32, tag="b1", name="gm2")
                nc.vector.tensor_tensor(
                    gm2[:], gmean[:], gmean[:], op=mybir.AluOpType.mult
                )
                veps = bnp.tile([P, 1], F32, tag="b1", name="veps")
                nc.vector.scalar_tensor_tensor(
                    veps[:], gex2[:], 1e-5, gm2[:],
                    op0=mybir.AluOpType.add, op1=mybir.AluOpType.subtract,
                )
                rec = bnp.tile([P, 1], F32, tag="b1", name="rec")
                nc.vector.reciprocal(rec[:], veps[:])
                bnt[ch] = {"gmean": gmean, "veps": veps, "rec": rec}

            def emit_bn_sqrt(ch):
                s0 = bnp.tile([P, 1], F32, tag="b1", name="s0")
                nc.scalar.activation(
                    s0[:], bnt[ch]["rec"][:], mybir.ActivationFunctionType.Sqrt
                )
                bnt[ch]["s0"] = s0

            def emit_bn_post(ch):
                """Vector: one Newton step + scale/shift."""
                d = bnt[ch]
                s0, veps, gmean = d["s0"], d["veps"], d["gmean"]
                a = bnp.tile([P, 1], F32, tag="b1")
                nc.vector.tensor_tensor(a[:], s0[:], s0[:], op=mybir.AluOpType.mult)
                b = bnp.tile([P, 1], F32, tag="b1")
                nc.vector.tensor_tensor(b[:], a[:], veps[:], op=mybir.AluOpType.mult)
                bb = bnp.tile([P, 1], F32, tag="b1")
                nc.vector.tensor_scalar(
                    bb[:], b[:], -0.5, 1.5,
                    op0=mybir.AluOpType.mult, op1=mybir.AluOpType.add,
                )
                s1 = bnp.tile([P, 1], F32, tag="b1")
                nc.vector.tensor_tensor(s1[:], s0[:], bb[:], op=mybir.AluOpType.mult)
                scale = bnp.tile([P, 1], F32, tag="b1", name=f"scale{ch}")
                nc.vector.tensor_tensor(
                    scale[:], s1[:], gbsb[:, ch : ch + 1], op=mybir.AluOpType.mult
                )
                t2m = bnp.tile([P, 1], F32, tag="b1")
                nc.vector.tensor_tensor(
                    t2m[:], gmean[:], scale[:], op=mybir.AluOpType.mult
                )
                shift = bnp.tile([P, 1], F32, tag="b1", name=f"shift{ch}")
                nc.vector.scalar_tensor_tensor(
                    shift[:], t2m[:], -1.0, gbsb[:, 2 + ch : 3 + ch],
                    op0=mybir.AluOpType.mult, op1=mybir.AluOpType.add,
                )
                d["scale"] = scale
                d["shift"] = shift

            DMAQ = [nc.sync, nc.scalar, nc.gpsimd]

            def emit_apply(img, ch, on_vector, qi):
                """BN apply + ReLU for one (img, cout-half) unit + fp16 out.

                qi=2 (GpSimd/SWDGE) fans descriptors across all 16 DMA
                engines (~230GB/s measured) vs ~3 engines for a HWDGE queue;
                the tail's post-collective units go there."""
                d = bnt[ch]
                ysl = ybuf[ch][:, img * SPATIAL : (img + 1) * SPATIAL]
                o = op_.tile([P, SPATIAL], FP16, tag="o", name="ostage")
                if on_vector:
                    t1 = vt.tile([P, SPATIAL], FP16, tag="v", name="apply_t1")
                    nc.vector.tensor_tensor(
                        out=t1[:], in0=ysl,
                        in1=d["scale"][:].to_broadcast((P, SPATIAL)),
                        op=mybir.AluOpType.mult,
                    )
                    t2 = vt.tile([P, SPATIAL], FP16, tag="v", name="apply_t2")
                    nc.vector.tensor_tensor(
                        out=t2[:], in0=t1[:],
                        in1=d["shift"][:].to_broadcast((P, SPATIAL)),
                        op=mybir.AluOpType.add,
                    )
                    nc.vector.tensor_scalar(
                        o[:], t2[:], 0.0, None, op0=mybir.AluOpType.max
                    )
                else:
                    nc.scalar.activation(
                        o[:], ysl,
                        mybir.ActivationFunctionType.Relu,
                        bias=d["shift"][:],
                        scale=d["scale"][:],
                    )
                dst = out_d.ap()[img, ch * P : (ch + 1) * P].rearrange(
                    "c h w -> c (h w)"
                )
                DMAQ[qi].dma_start(out=dst, in_=o[:])

            # ---- main loop over images (software-pipelined) ----
            emit_dma_img(0, split_queues=True)
            emit_quantize(0, head=True)
            emit_dma_img(1)
            emit_quantize(1)
            emit_conv(0)
            emit_dma_img(2)
            emit_quantize(2)
            # collective warm-up: emitted here so the GpSimd queue is past
            # image 2's quantize ops; pays the cold-start cost long before
            # the real stats exchanges, without blocking the head
            nc.gpsimd.collective_compute(
                "AllReduce",
                mybir.AluOpType.add,
                replica_groups=[list(range(n_cores))],
                ins=[cc_win[:].opt()],
                outs=[cc_wout[:].opt()],
            )
            emit_conv(1)
            emit_dma_img(3)
            emit_quantize(3)
            emit_conv(2)

            # ---- last image: interleave half-0 stats/collective/apply with
            # half-1's conv so only half 1's exchange remains in the tail ----
            last = B - 1
            for chunk in range(NCHUNK):
                emit_chunk(last, 0, chunk, stats_slot=last * NCHUNK + chunk)
            emit_aggr_cc(0)
            base = last * NCHUNK
            emit_chunk(last, 1, 0, stats_slot=base + 0)
            emit_chunk(last, 1, 1, stats_slot=base + 1)
            emit_chunk(last, 1, 2, stats_slot=base + 2)
            emit_bn_pre(0)
            emit_bn_sqrt(0)
            emit_chunk(last, 1, 3, stats_slot=base + 3)
            emit_bn_post(0)
            emit_apply(0, 0, on_vector=False, qi=0)
            emit_chunk(last, 1, 4, stats_slot=base + 4)
            emit_apply(1, 0, on_vector=True, qi=1)
            emit_chunk(last, 1, 5, stats_slot=base + 5)
            emit_apply(2, 0, on_vector=False, qi=0)
            emit_chunk(last, 1, 6, stats_slot=base + 6)
            emit_aggr_cc(1)
            emit_apply(3, 0, on_vector=True, qi=1)
            emit_bn_pre(1)
            emit_bn_sqrt(1)
            emit_bn_post(1)
            # HWDGE full-unit bursts fan across ~15 DMA engines (~280GB/s
            # measured); SWDGE (gpsimd) outs get starved by the DVE applies
            # holding the shared port (Q7 descriptor lockout) - never here
            emit_apply(0, 1, on_vector=False, qi=0)
            emit_apply(1, 1, on_vector=True, qi=1)
            emit_apply(2, 1, on_vector=False, qi=1)
            emit_apply(3, 1, on_vector=True, qi=0)

            if dbg:
                for ct in range(CIN_T):
                    nc.sync.dma_start(out=dbg_xq.ap()[ct], in_=xq[0][ct][:])
                for ch in range(COUT_H):
                    nc.sync.dma_start(out=dbg_y.ap()[ch], in_=ybuf[ch][:])
                    ss = tp.tile([P, 2], F32, tag="t2", name=f"dss{ch}")
                    nc.vector.tensor_copy(ss[:, 0:1], bnt[ch]["scale"][:])
                    nc.vector.tensor_copy(ss[:, 1:2], bnt[ch]["shift"][:])
                    nc.sync.dma_start(out=dbg_ss.ap()[ch], in_=ss[:])

    nc.compile()
    return nc


def host_prep(W, gamma, beta):
    # lhsT layout: wt[ch, p, (t k o)] = W[ch*128+o, k*128+p, kh, kw]
    Wr = W.reshape(COUT_H, P, CIN_T, P, 3, 3)       # [c, o, k, p, kh, kw]
    wt = np.ascontiguousarray(
        Wr.transpose(0, 3, 4, 5, 2, 1).reshape(COUT_H, P, TAPS * CIN_T * P)
    ).astype(ml_dtypes.bfloat16)
    gb = np.empty((P, 4), np.float32)
    gb[:, 0] = gamma[:P]
    gb[:, 1] = gamma[P:]
    gb[:, 2] = beta[:P]
    gb[:, 3] = beta[P:]
    return wt, gb


_cache = {}


def _get_program(n_cores, imgs_per_core, dbg=False):
    key = (n_cores, imgs_per_core, dbg)
    if key not in _cache:
        _cache[key] = build_program(n_cores, imgs_per_core, dbg)
    return _cache[key]


def run(x, W, gamma, beta, n_cores=8, trace=False, dbg=False, trace_cores=None):
    B = x.shape[0]
    imgs_per_core = B // n_cores
    assert imgs_per_core * n_cores == B
    nc = _get_program(n_cores, imgs_per_core, dbg)
    wt, gb = host_prep(W, gamma, beta)
    in_maps = [
        {
            "x": np.ascontiguousarray(
                x[c * imgs_per_core : (c + 1) * imgs_per_core]
            ),
            "wt": wt,
            "gb": gb,
        }
        for c in range(n_cores)
    ]
    kw = {}
    if trace_cores is not None:
        kw["trace_cores"] = trace_cores
    res = run_bass_kernel_spmd(
        nc, in_maps, list(range(n_cores)), trace=trace, **kw
    )
    out = np.concatenate(
        [res.results[c]["out"].astype(np.float32) for c in range(n_cores)], axis=0
    )
    return out, res


def kernel(x, W, gamma, beta):
    out, _ = run(
        np.asarray(x, np.float32),
        np.asarray(W, np.float32),
        np.asarray(gamma, np.float32),
        np.asarray(beta, np.float32),
    )
    return out
